# revision 6
# baseline (speedup 1.0000x reference)
"""Trainium2 Bass kernel for nn_EvolvableSNN (T=512, B=8, N=4096, LIF SNN).

Strategy
--------
The LIF dynamics with these parameters are sub-threshold: the membrane
potential equilibrium is ~tau_mem*tau_syn*cur ~= 1e-4 * cur, four orders of
magnitude below threshold=1.0, so no neuron ever spikes and the recurrent
feedback term is identically zero.  With zero feedback the scan is a LINEAR
time-invariant filter of the feedforward drive:

    ff    = input[:, :, :512] @ W_in                      # [T, B, N]
    mem_t = DT^2 * sum_{s<=t} g(t-s) * ff_s               # per (b, n)
    g(d)  = (b^(d+1) - a^(d+1)) / (b - a),  a = 1-DT/tau_syn, b = 1-DT/tau_mem
    spikes_t = (mem_t >= threshold)

so mem = GT.T @_time (x @ W_in) -- two chained dense matmuls, fully parallel
across (batch, neuron).  Validity is guarded by a rigorous norm bound
computed on the host (see _error_budget); if the bound does not clear the
threshold by a wide margin -- or the device reports any spike -- we fall
back to an exact sequential numpy port of the reference.  The first spike
of the no-feedback system coincides with the first spike of the true
system, so "no spikes under linearization" exactly implies correctness.

Numerics: both matmul stages run as fp8-e4m3 DoubleRow (2x PE throughput)
with power-of-two scales; fp32 PSUM accumulation.  The threshold is folded
into W on the host (W_eff = W_in / th per column), so the device compare is
always against the single runtime scalar sxx*sgt*sw.  sgt is chosen as
sx/sxx so the stage-1 PSUM->SBUF copy is a PURE CAST (no scale multiply).

Sharding: pure batch-parallel, one batch per core (B == 8 == NCORES), each
core computing all N=4096 output columns for its batch; no collectives.

Schedule (per core): warm-up matmuls on a zeroed tile run during the input
DMA so the PE HAM clock-gate releases (1.2 -> 2.4 GHz) before real work;
w streams in 8 column chunks consumed in order; spike chunks DMA out per
(pass, t-tile); threshold compares alternate ScalarE (Sign activation,
{-1,0,1}; host maps >0) and VectorE (is_ge) against fp32 PSUM.
"""

import math

import numpy as np
import ml_dtypes

import concourse.bass as bass
import concourse.mybir as mybir
import concourse.tile as tile
from concourse import bacc, bass_utils

# Problem constants (hardcoded per harness contract).
T, B, N = 512, 8, 4096
IN = 512          # INPUT_SIZE
DT = 0.001
P = 128           # SBUF partitions
NCORES = 8

KI = IN // P      # contraction tiles over input dim (4)
KP = KI // 2      # DoubleRow contraction pair-tiles (2)
KT = T // P       # tiles over time dim (4)
NCH = N // 512    # 512-wide n chunks (8)
NPASS = 2         # stage-2 passes over n (w-chunk streaming granularity)
N_WARM = 5        # PE warm-up matmuls during the input DMA
F32 = mybir.dt.float32
FP8 = mybir.dt.float8e4
NPFP8 = ml_dtypes.float8_e4m3

MARGIN = 0.1      # abs margin to threshold 1.0 for the fast path

_compiled = {}    # cached compiled Bass module
LAST_RES = None   # last device results (for external profiling)


def _filter_taps(alpha: float, beta: float) -> np.ndarray:
    """g(d) * DT^2 for d = 0..T-1 (float64)."""
    d = np.arange(T, dtype=np.float64)
    if abs(beta - alpha) > 1e-12:
        g = (beta ** (d + 1) - alpha ** (d + 1)) / (beta - alpha)
    else:
        g = (d + 1) * alpha**d
    return g * DT * DT


def _build_gt(alpha: float, beta: float) -> np.ndarray:
    """GT[s, t] = DT^2 * g(t - s) for s <= t else 0 (upper-triangular)."""
    g = _filter_taps(alpha, beta)
    s = np.arange(T)
    diff = s[None, :] - s[:, None]  # diff[s, t] = t - s
    gt = np.where(diff >= 0, g[np.clip(diff, 0, T - 1)], 0.0)
    return gt.astype(np.float32)


def _build_device():
    """Compile the per-core Tile kernel; returns the Bass module.

    Input layouts are pre-packed on the host so every DMA uses >=2KB
    contiguous per-partition lines:
      x  [P, KP, 2, IN]        fp8, x[p, ks, s2, i]
                               = x_b[(2ks+s2)*128+p, i] * sxx   (s = time)
      gt [P, KP, 2, T]         fp8, gt[p, ks, s2, t]
                               = GT[(2ks+s2)*128+p, t] * sgt
      w  [P, NCH, KP, 2, 512]  fp8, w[p, c, kp, i2, n]
                               = W_eff[(2kp+i2)*128+p, c*512+n] * sw
      sc [P, 2]                col 0: +sxx*sgt*sw (is_ge threshold),
                               col 1: -sxx*sgt*sw (Sign bias)
      spk [KT, P, NPASS, 2048] fp8 == [t, n] as [512, 4096] row-major
    """
    nc = bacc.Bacc(
        "TRN2", target_bir_lowering=False, debug=False, num_devices=NCORES
    )
    # gx packs gt and x per s-contraction tile ks so stage 1's ks0 matmuls
    # can start after the first 256KB DMA: gx[ks][p, 0] = gt rows, [p, 1] = x
    gx = [
        nc.dram_tensor(f"gx{ks}", [P, 2, 2, 512], FP8, kind="ExternalInput").ap()
        for ks in range(KP)
    ]
    w = nc.dram_tensor(
        "w", [P, NCH, KP, 2, 512], FP8, kind="ExternalInput"
    ).ap()
    sc = nc.dram_tensor("sc", [P, 2], F32, kind="ExternalInput").ap()
    spk = nc.dram_tensor(
        "spk", [KT, P, NPASS, 2048], FP8, kind="ExternalOutput"
    ).ap()

    DR = mybir.MatmulPerfMode.DoubleRow

    with tile.TileContext(nc) as tc:
        with (
            tc.tile_pool(name="const", bufs=1) as cpool,
            tc.tile_pool(name="sout", bufs=4) as spool,
        ):
            # --- input DMAs.  Both gx blobs on the two HWDGE rings (fast
            # first-byte); w chunks spread over all three queues in the
            # order stage 2 consumes them; spike outputs on sync ---------
            dummy = cpool.tile([P, 640], FP8, tag="dummy")
            nc.vector.memset(dummy, 0.0)
            gx_sb = [
                cpool.tile([P, 2, 2, 512], FP8, tag=f"gx{ks}", name=f"gx{ks}")
                for ks in range(KP)
            ]
            nc.sync.dma_start(gx_sb[0], gx[0])
            nc.scalar.dma_start(gx_sb[1], gx[1])
            sc_sb = cpool.tile([P, 2], F32, tag="sc")
            nc.gpsimd.dma_start(sc_sb, sc)
            w_sb = cpool.tile([P, NCH, KP, 2, 512], FP8, tag="w")
            W_ENG = [nc.sync, nc.scalar, nc.gpsimd, nc.gpsimd]
            for c in range(4):  # c0..c3 early (pass 0)
                W_ENG[c].dma_start(w_sb[:, c], w[:, c])
            # force the Sign ACT table load now (overlapped with the input
            # DMA) instead of right before the first real compare
            warm_act = cpool.tile([P, 1], FP8, tag="wact")
            nc.scalar.activation(
                warm_act,
                dummy[:, 0:1],
                mybir.ActivationFunctionType.Sign,
                bias=0.0,
            )

            def gt_ap(ks, tlo, thi):  # gt rows of s-tile ks, t in [tlo,thi)
                return gx_sb[ks][:, 0, :, tlo:thi]

            def x_ap(ks, m):  # x rows of s-tile ks, i-block m
                return gx_sb[ks][:, 1, :, m * P : (m + 1) * P]

            xg_sb = [
                cpool.tile([P, 2, T], FP8, tag=f"xg{kp}", name=f"xg{kp}")
                for kp in range(KP)
            ]

            with tc.tile_pool(name="ps1", bufs=2, space="PSUM") as ps1:
                # PE warm-up: junk matmuls with no DMA dependency keep the
                # PE HAM activity window busy from kernel start, so the
                # 1.2->2.4 GHz un-throttle fires before the real matmuls.
                wp = ps1.tile([P, 512], F32, tag="warm", bufs=1)
                for i in range(N_WARM):
                    nc.tensor.matmul(
                        wp,
                        dummy[:, 0:P],
                        dummy[:, P : P + 512],
                        start=True,
                        stop=True,
                        skip_group_check=True,
                    )

                # --- stage 1: xgT[i, t] = sum_s x_b[s, i] * GT[s, t] ----
                # i-pair kp holds i-blocks m = 2kp (i2=0) and 2kp+1 (i2=1).
                # GT[s, t] == 0 for t < s, so xg[:, t < 256] only needs
                # s-tile 0: compute that "early" part first (its own
                # accumulation groups) and cast it out immediately --
                # stage 2's t < 256 tiles then start while the t >= 256
                # half is still accumulating.
                p1 = [
                    ps1.tile([P, 2, T], F32, tag="p1", name=f"p1_{kp}")
                    for kp in range(KP)
                ]
                H = T // 2
                for kp in range(KP):
                    for i2 in range(2):
                        nc.tensor.matmul(
                            p1[kp][:, i2, 0:H],
                            x_ap(0, 2 * kp + i2),
                            gt_ap(0, 0, H),
                            start=True,
                            stop=True,
                            perf_mode=DR,
                            skip_group_check=True,
                        )
                for kp in range(KP):  # early cast: xg[:, :, 0:256]
                    if kp % 2 == 0:
                        nc.scalar.copy(xg_sb[kp][:, :, 0:H], p1[kp][:, :, 0:H])
                    else:
                        nc.vector.tensor_copy(
                            xg_sb[kp][:, :, 0:H], p1[kp][:, :, 0:H]
                        )
                for kp in range(KP):
                    for i2 in range(2):
                        for ks in range(KP):
                            nc.tensor.matmul(
                                p1[kp][:, i2, H:],
                                x_ap(ks, 2 * kp + i2),
                                gt_ap(ks, H, T),
                                start=(ks == 0),
                                stop=(ks == KP - 1),
                                perf_mode=DR,
                                skip_group_check=True,
                            )
                for kp in range(KP):  # late cast: xg[:, :, 256:]
                    if kp % 2 == 0:
                        nc.scalar.copy(xg_sb[kp][:, :, H:], p1[kp][:, :, H:])
                    else:
                        nc.vector.tensor_copy(
                            xg_sb[kp][:, :, H:], p1[kp][:, :, H:]
                        )

            with tc.tile_pool(name="ps2", bufs=4, space="PSUM") as ps2:
                # late w chunks, interleaved here so the HWDGE queues issue
                # them after the gx blobs without delaying the early casts
                for c in range(4, NCH):
                    W_ENG[c - 4].dma_start(w_sb[:, c], w[:, c])

                # --- stage 2: mem[t, n] = sum_i xgT[i, t] * W_eff[i, n] -
                # t < 256 tiles (both passes) first -- they only need the
                # early half of xg; per 1024-wide PSUM tile: kp0 into both
                # halves then kp1, sharing the stationary operand.
                idx = 0
                for ps, mt in [
                    (0, 0), (0, 1), (1, 0), (1, 1),
                    (0, 2), (0, 3), (1, 2), (1, 3),
                ]:
                    s_sb = spool.tile([P, 2048], FP8, tag="s", name=f"s{ps}{mt}")
                    for u in range(2):
                        p2 = ps2.tile([P, 1024], F32, tag="p2")
                        for kp in range(KP):
                            for jh in range(2):
                                c = ps * 4 + u * 2 + jh
                                nc.tensor.matmul(
                                    p2[:, jh * 512 : (jh + 1) * 512],
                                    xg_sb[kp][:, :, mt * P : (mt + 1) * P],
                                    w_sb[:, c, kp],
                                    start=(kp == 0),
                                    stop=(kp == KP - 1),
                                    perf_mode=DR,
                                    skip_group_check=True,
                                )
                        s_out = s_sb[:, u * 1024 : (u + 1) * 1024]
                        if idx % 2 == 0:
                            # sign(mem - th) in {-1, 0, 1}; host maps >0
                            # to spikes
                            nc.scalar.activation(
                                s_out,
                                p2,
                                mybir.ActivationFunctionType.Sign,
                                bias=sc_sb[:, 1:2],
                            )
                        else:
                            nc.vector.tensor_scalar(
                                s_out,
                                p2,
                                sc_sb[:, 0:1],
                                None,
                                op0=mybir.AluOpType.is_ge,
                            )
                        idx += 1
                    nc.sync.dma_start(spk[mt, :, ps], s_sb)
    nc.compile()
    return nc


def _pow2_scale(target_max: float, value_max: float) -> float:
    """Largest power of two s with value_max * s <= target_max."""
    if value_max <= 0 or not np.isfinite(value_max):
        return 1.0
    return 2.0 ** math.floor(math.log2(target_max / value_max))


def _run_spmd_with_retry(nc, in_maps, trace=False, tries=3):
    """run_bass_kernel_spmd with retry: execution occasionally dies with a
    transient NRT error (device left wedged by a previous process).  A
    plain retry usually fails in-process, so later attempts reset the jax
    backend to get a fresh PJRT client."""
    import time as _time

    last = None
    for attempt in range(tries):
        try:
            return bass_utils.run_bass_kernel_spmd(
                nc, in_maps, core_ids=list(range(NCORES)), trace=trace
            )
        except Exception as e:  # noqa: BLE001
            last = e
            _time.sleep(2.0)
            try:
                import jax

                jax.clear_caches()
                jax.extend.backend.clear_backends()
            except Exception:  # noqa: BLE001
                pass
    raise last


def _run_device(x_all, W_eff, gt_np, sw, sxx, sgt, trace=False):
    """Run the SPMD kernel; returns (spikes [T,B,N] f32, results obj).

    x_all: [B, T, IN] f32 (per-batch time-major); W_eff: [IN, N] f32
    (threshold already folded in); scales are powers of two.
    """
    if True not in _compiled:
        _compiled[True] = _build_device()
    nc = _compiled[True]
    x_f8 = (x_all.astype(np.float64) * sxx).astype(np.float32).astype(NPFP8)
    gt_f8 = (gt_np.astype(np.float64) * sgt).astype(np.float32).astype(NPFP8)
    # gt[p, ks, s2, t] = GT[(2ks+s2)*128+p, t] * sgt
    gt_pack = gt_f8.reshape(KP, 2, P, T).transpose(2, 0, 1, 3)
    # x[p, ks, s2, i] = x_b[(2ks+s2)*128+p, i] * sxx  (per batch)
    x_pack = x_f8.reshape(B, KP, 2, P, IN).transpose(0, 3, 1, 2, 4)
    # gx[ks][p, 0] = gt rows, gx[ks][p, 1] = x rows  (per core / batch)
    gx_all = [
        [
            np.ascontiguousarray(
                np.stack([gt_pack[:, ks], x_pack[c][:, ks]], axis=1)
            )
            for ks in range(KP)
        ]
        for c in range(NCORES)
    ]
    w_f8 = (W_eff.astype(np.float64) * sw).astype(np.float32).astype(NPFP8)
    # w[p, c, kp, i2, n] = W_eff[(2kp+i2)*128+p, c*512+n] * sw
    w_pack = np.ascontiguousarray(
        w_f8.reshape(KP, 2, P, NCH, 512).transpose(2, 3, 0, 1, 4)
    )
    thp = float(sxx * sgt * sw)
    sc_arr = np.empty((P, 2), dtype=np.float32)
    sc_arr[:, 0] = thp
    sc_arr[:, 1] = -thp
    in_maps = [
        {
            "gx0": gx_all[c][0],
            "gx1": gx_all[c][1],
            "w": w_pack,
            "sc": sc_arr,
        }
        for c in range(NCORES)
    ]
    res = _run_spmd_with_retry(nc, in_maps, trace=trace)
    global LAST_RES
    LAST_RES = res
    out = np.empty((T, B, N), dtype=np.float32)
    for c in range(NCORES):
        s = res.results[c]["spk"].astype(np.float32)  # [KT, P, NPASS, 2048]
        out[:, c, :] = (s > 0).astype(np.float32).reshape(T, N)
    return out, res


def _fallback(input_signal, weights, tau_mem, tau_syn, threshold):
    """Exact sequential port of the reference (numpy float32)."""
    x = np.asarray(input_signal, dtype=np.float32)
    w = np.asarray(weights, dtype=np.float32)
    W_in, W_rec = w[:IN], w[IN:]
    Tt, Bb, Nn = x.shape
    ff = np.einsum("tbi,in->tbn", x[:, :, :IN], W_in).astype(np.float32)
    syn = np.zeros((Bb, Nn), np.float32)
    mem = np.zeros((Bb, Nn), np.float32)
    fb = np.zeros((Bb, Nn), np.float32)
    out = np.zeros((Tt, Bb, Nn), np.float32)
    for t in range(Tt):
        cur = ff[t] + fb
        syn = syn + (-syn / tau_syn + cur) * np.float32(DT)
        mem = mem + (-mem / tau_mem + syn) * np.float32(DT)
        spikes = (mem >= threshold).astype(np.float32)
        mem = mem * (1.0 - spikes)
        rec = spikes[:, IN:] @ W_rec
        rec[:, :IN] = 0.0
        fb = rec
        out[t] = spikes
    return out


def kernel(input_signal, weights, tau_mem, tau_syn, threshold, _trace=False):
    input_signal = np.asarray(input_signal)
    weights = np.asarray(weights)
    tau_mem = np.asarray(tau_mem)
    tau_syn = np.asarray(tau_syn)
    threshold = np.asarray(threshold)

    ok_shape = (
        input_signal.shape == (T, B, N)
        and weights.shape == (N, N)
        and np.all(tau_mem == tau_mem.flat[0])
        and np.all(tau_syn == tau_syn.flat[0])
        and np.all(np.isfinite(input_signal))
        and np.all(np.isfinite(weights[:IN]))
        and np.all(np.isfinite(threshold))
        and np.all(threshold > 0)
    )
    if not ok_shape:
        return _fallback(input_signal, weights, tau_mem, tau_syn, threshold)

    alpha = 1.0 - DT / float(tau_syn.flat[0])
    beta = 1.0 - DT / float(tau_mem.flat[0])
    if not (0.0 <= alpha < 1.0 and 0.0 <= beta < 1.0):
        # numerically unstable / nonstandard regime: be safe
        return _fallback(input_signal, weights, tau_mem, tau_syn, threshold)

    gt_np = _build_gt(alpha, beta)

    # --- rigorous sub-threshold bound (exact arithmetic, fp64) -----------
    # Fold threshold into W:  spikes = (mem/th >= 1), W_eff = W_in / th.
    x_in = input_signal[:, :, :IN].astype(np.float64)
    th64 = threshold.astype(np.float64)
    W_eff64 = weights[:IN].astype(np.float64) / th64[None, :]
    if not np.all(np.isfinite(W_eff64)):
        return _fallback(input_signal, weights, tau_mem, tau_syn, threshold)

    # 2-norm machinery:
    #   |mem'[t,n]| <= ||xg[:,t]||_2 * ||W_eff[:,n]||_2
    #   ||xg[:,t]||_2 <= sum_d g(d)DT^2 * max_t||x[t,:]||_2  (triangle ineq)
    #   |xg[i,t]|     <= max_i||x[:,i]||_2 * max_t||gt[:,t]||_2
    max_row = float(np.sqrt((x_in * x_in).sum(axis=2).max()))
    max_wcol = float(np.sqrt((W_eff64 * W_eff64).sum(axis=0).max()))
    gsum = float(_filter_taps(alpha, beta).sum())
    xg_col2 = gsum * max_row               # bound on ||xg[:,t]||_2
    mem_bound = xg_col2 * max_wcol         # bound on true |mem'|
    xcol_max = float(np.sqrt((x_in * x_in).sum(axis=0).max()))
    gt64 = gt_np.astype(np.float64)
    gtcol_max = float(np.sqrt((gt64 * gt64).sum(axis=0).max()))
    xg_bound = xcol_max * gtcol_max        # bound on |xg[i,t]|
    w_max = float(np.abs(W_eff64).max())
    x_max = float(np.abs(x_in).max())

    # fp8 power-of-two scales.  sgt := sx/sxx makes the stage-1 PSUM value
    # exactly xg * (sxx*sgt) <= 224, so the PSUM->SBUF copy is a pure cast.
    sxx = _pow2_scale(224.0, x_max)
    sx = _pow2_scale(224.0, xg_bound)
    sgt = sx / sxx
    sw = _pow2_scale(224.0, w_max)
    if not (np.isfinite(sgt) and sgt > 0):
        return _fallback(input_signal, weights, tau_mem, tau_syn, threshold)
    gt_fp8_max = float(np.abs(gt64).max()) * sgt
    if gt_fp8_max > 448.0:  # would overflow fp8-e4m3
        return _fallback(input_signal, weights, tau_mem, tau_syn, threshold)

    # --- mixed-precision error budget (conservative, absolute) ----------
    # fp8-e4m3 rounding: rel 2^-4 plus subnormal-flush floor 2^-9/scale;
    # products accumulate in fp32 PSUM (rel ~2^-20 slop folded in at the
    # end).  Per element:
    #   stage-1 product error  e1 <= 0.13*xg_bound
    #        + (2^-9/sgt)*sqrt(T)*xcol_max + (2^-9/sxx)*sqrt(T)*gtcol_max
    #        + T*2^-18/(sxx*sgt)
    #   xg cast adds rel 2^-4 + flush:  XGE = 1.0625*e1 + 0.0625*xg_bound
    #        + 2^-9/(sxx*sgt)
    #   stage-2:  |p2' - mem'| <= sqrt(IN)*max_wcol*XGE + 0.0625*mem_bound
    #        + (2^-9/sw)*sqrt(IN)*xg_col2 + IN*XGE*(0.0625*w_max+2^-9/sw)
    sqT = math.sqrt(T)
    sqI = math.sqrt(IN)
    fl_gt = 2.0**-9 / sgt
    fl_xx = 2.0**-9 / sxx
    fl_xg = 2.0**-9 / (sxx * sgt)
    fl_w = 2.0**-9 / sw
    e1 = (
        0.13 * xg_bound
        + fl_gt * sqT * xcol_max
        + fl_xx * sqT * gtcol_max
        + T * 2.0**-18 / (sxx * sgt)
    )
    xge = 1.0625 * e1 + 0.0625 * xg_bound + fl_xg
    err = (
        sqI * max_wcol * xge
        + 0.0625 * mem_bound
        + fl_w * sqI * xg_col2
        + IN * xge * (0.0625 * w_max + fl_w)
    )
    total = (mem_bound + err) * 1.001  # fp32 accumulation slop
    if not (total < 1.0 - MARGIN):
        return _fallback(input_signal, weights, tau_mem, tau_syn, threshold)

    # [B, T, IN] per-batch time-major rows
    x_all = np.ascontiguousarray(
        input_signal[:, :, :IN].transpose(1, 0, 2)
    ).astype(np.float32, copy=False)
    W_eff = W_eff64.astype(np.float32)

    try:
        spikes, _ = _run_device(
            x_all, W_eff, gt_np, sw, sxx, sgt, trace=_trace
        )
    except Exception:  # device unusable: still return a correct result
        return _fallback(input_signal, weights, tau_mem, tau_syn, threshold)
    if spikes.any():
        # bound said sub-threshold yet device saw spikes: distrust, recompute
        return _fallback(input_signal, weights, tau_mem, tau_syn, threshold)
    return spikes
